# revision 1
# baseline (speedup 1.0000x reference)
"""Banded DTW loss kernel for Trainium2 (Bass/Tile), 8-core data-parallel.

Algorithm (per sample, N=1024, Sakoe-Chiba half-width W=20, band width 41):
  Phase A: forward DP row-by-row. Row recurrence
             D[i,j] = d[i,j] + min(D[i-1,j-1], D[i-1,j], D[i,j-1])
           is computed with ONE tensor_tensor_scan per row
           (state = min(d + state, min(diag,up)+d)), bit-identical values
           to the reference wavefront DP. D rows stream through a rolling
           16-row window and are DMA'd into a [128,*]-partition RE layout.
  Phase B: per-cell backtrack choice bits (argmin with diag>up>left
           preference, replicating the reference bt_step exactly), then a
           per-row scan producing g[row, col] = entry column of row-1 when
           the backtrack enters `row` at `col`.
  Phase C: walk rows 1023..1 with ONE scalar_tensor_tensor per row
           (one-hot extract of g at the current column; accum_out = next
           column).
  Phase D: the path covers a contiguous column interval per row; build
           interval masks and reduce the four path aggregates
           (sum|dx|, sum|dy|, sum bce, count) with big parallel ops.

All compute ops keep every SBUF operand at the same start partition
(0 or 64) to satisfy the birverifier's samePartitionsAll check.

Sharding: batch 32 -> 4 samples per core on 8 cores; host sums partials.
"""

import numpy as np

import concourse.bacc as bacc
import concourse.bass as bass
import concourse.mybir as mybir
import concourse.tile as tile
from concourse.bass_utils import run_bass_kernel_spmd

B, N, NF = 32, 1024, 4
W = 20
NCORES = 8
BC = B // NCORES          # samples per core
BIG = 1e30
NB = 41                   # band width (o = j - i + 20 in [0,40])
CW = 43                   # RE row width (col 0 pad, col c=o+1, col 42 pad)
NBLK = 33                 # RE blocks (r = i+1 in [0,1024], p=r%32, b=r//32)
PPAD_T = 1056             # ppad time length  (ppad[:,1+i,:] = preds[:,i,:])
TPAD_T = 1100             # tpad time length  (tpad[:,21+i,:] = targs[:,i,:])
SKW = 1066                # skewed targ row length
NRING = 16

AL = mybir.AluOpType
DT = mybir.dt.float32

NWIN = 64                 # rolling window depth (rows), ring ditto

# ---- megaQ ([128, QW]) column offsets; quadrant shadows @0 and @64 ----
WIN_O = 0                 # @0: rolling D window, 64 slots * 42 (col 41 BIG)
VR_O = WIN_O + NWIN * 42             # @0: virtual row r=0 (42 cols)
RING_O = VR_O + 42                   # @0: d ring, 64 slots * 41
TMP_O = RING_O + NWIN * NB           # @0: phase-A tmp
DA1_O = TMP_O + 48                   # @0: phase-A data1
WSCL_O = DA1_O + 48                  # @0: walk scratch (lo half)
XHL_O = WSCL_O + 48                  # @0: xhist cols i in [0,512)
GWLO_O = XHL_O + 512                 # @0: g rows i in [0,512), 41 each
QW_LO = GWLO_O + 512 * NB
GWHI_O = 0                # @64: g rows i in [512,1024)
XHH_O = GWHI_O + 512 * NB            # @64: xhist col (i-511), i in [511,1024)
WSCH_O = XHH_O + 513                 # @64: walk scratch (hi half)
QW = max(QW_LO, WSCH_O + 48)

# ---- megaRE ([128, 8*1419 + 448]) regions ----
RE = NBLK * CW            # 1419
R1_O, R2_O, R3_O, R4_O, R5_O, R6_O, R7_O, R8_O = (i * RE for i in range(8))
SM_O = 8 * RE             # small-tensor block (448 cols)
PX_O, PY_O, PZ_O = SM_O, SM_O + 33, SM_O + 66
XC_O, OLO_O = SM_O + 99, SM_O + 132
PCONST_O, COLIO_O = SM_O + 165, SM_O + 166   # colio values 0..42
CLZ_O, SPZ_O, SPN_O, QZ_O, NGZ_O = (SM_O + c for c in (209, 242, 275, 308, 341))
RED_O = SM_O + 374        # Sx, Sy, Sbce, cnt
ROWC_O = SM_O + 378       # per-(p,b) row constant 32b + p - 22
REW = SM_O + 448

_CACHE = {}


def _manual_ap(base, extra_off, dims):
    """AP with base's partition dim and explicit free [stride, count] dims."""
    ap0 = [list(base.ap[0])]
    return bass.AP(base.tensor, base.offset + extra_off,
                   ap0 + [list(d) for d in dims])


def _build_module():
    nc = bacc.Bacc("TRN2", target_bir_lowering=False, debug=False,
                   num_devices=NCORES)
    pre = nc.dram_tensor("pre", [128, 99], DT, kind="ExternalInput")
    tsk = nc.dram_tensor("tsk", [128, 3 * SKW], DT, kind="ExternalInput")
    cst = nc.dram_tensor("cst", [128, 78], DT, kind="ExternalInput")
    partials = nc.dram_tensor("partials", [128, 4], DT, kind="ExternalOutput")
    dram_d = nc.dram_tensor("dscr_d", [BC, N, NB], DT, kind="Internal")
    dram_D = nc.dram_tensor("dscr_D", [BC, N, NB], DT, kind="Internal")

    with tile.TileContext(nc) as tc:
        with tc.tile_pool(name="main", bufs=1) as pool:
            megaQ = pool.tile([128, QW], DT)
            megaRE = pool.tile([128, REW], DT)
            _emit(nc, megaQ, megaRE, pre, tsk, cst, partials, dram_d, dram_D)
    nc.compile()
    return nc


def _emit(nc, megaQ, megaRE, pre, tsk, cst, partials, dram_d, dram_D):
    v = nc.vector

    def cells(off, dc=0):
        """[128, 33, 41] view of RE region cols (b*43 + 1 + dc)."""
        return megaRE[:, off:off + RE].rearrange(
            "p (b c) -> p b c", c=CW)[:, :, 1 + dc:NB + 1 + dc]

    def reblk(off, b, dc=0, w=NB):
        s = off + b * CW + 1 + dc
        return megaRE[:, s:s + w]

    def smb(off):
        """[128, 33] small block broadcast over the 41 band cols."""
        return megaRE[:, off:off + NBLK].unsqueeze(2).broadcast_to([128, NBLK, NB])

    def skwin(off):
        """skewed targ window [128, 33, 41]: u = 32b + (c-1)."""
        base = megaRE[:, off:off + 1]
        return _manual_ap(base, 0, [[32, NBLK], [1, NB]])

    # ---------------- input DMAs (host pre-laid-out) ----------------
    nc.sync.dma_start(out=megaRE[:, PCONST_O:PCONST_O + 44], in_=cst[:, 0:44])
    nc.sync.dma_start(out=megaRE[:, ROWC_O:ROWC_O + NBLK], in_=cst[:, 44:44 + NBLK])
    nc.sync.dma_start(out=megaRE[:, PX_O:PX_O + 99], in_=pre[:])
    for k, off in ((0, R1_O), (1, R2_O), (2, R3_O)):
        for c0, c1 in ((0, 267), (267, 534), (534, 800), (800, SKW)):
            nc.sync.dma_start(out=megaRE[:, off + c0:off + c1],
                              in_=tsk[:, k * SKW + c0:k * SKW + c1])

    # ---------------- d build (all cells, RE layout) ----------------
    ocolv = megaRE[:, COLIO_O + 1:COLIO_O + 1 + NB].unsqueeze(1) \
        .broadcast_to([128, NBLK, NB])
    # jmap = (32b + p - 22) + oc  (the j index of each band cell)
    v.tensor_tensor(out=cells(R5_O), in0=smb(ROWC_O), in1=ocolv, op=AL.add)
    v.tensor_single_scalar(out=cells(R6_O), in_=cells(R5_O),
                           scalar=0.0, op=AL.is_ge)
    v.tensor_single_scalar(out=cells(R7_O), in_=cells(R5_O),
                           scalar=float(N - 1), op=AL.is_le)
    v.tensor_tensor(out=cells(R6_O), in0=cells(R6_O), in1=cells(R7_O), op=AL.mult)
    # vmb = BIG at invalid cells, 0 at valid ones
    v.tensor_scalar(out=cells(R7_O), in0=cells(R6_O),
                    scalar1=-BIG, scalar2=BIG, op0=AL.mult, op1=AL.add)
    # |dx|, |dy| for every cell (also the Sx/Sy metric inputs)
    v.tensor_tensor(out=cells(R5_O), in0=smb(PX_O), in1=skwin(R1_O),
                    op=AL.subtract)
    v.scalar_tensor_tensor(out=cells(R1_O), in0=cells(R5_O), scalar=-1.0,
                           in1=cells(R5_O), op0=AL.mult, op1=AL.max)
    v.tensor_tensor(out=cells(R5_O), in0=smb(PY_O), in1=skwin(R2_O),
                    op=AL.subtract)
    v.scalar_tensor_tensor(out=cells(R2_O), in0=cells(R5_O), scalar=-1.0,
                           in1=cells(R5_O), op0=AL.mult, op1=AL.max)
    v.tensor_tensor(out=cells(R5_O), in0=cells(R1_O), in1=cells(R2_O), op=AL.add)
    # dcost = max(d, vmb): exactly d at valid cells, exactly BIG at invalid
    v.tensor_tensor(out=cells(R6_O), in0=cells(R5_O), in1=cells(R7_O), op=AL.max)

    # ---------------- stage dcost to DRAM (per-p, before phase A) ----------------
    df = dram_d[:]
    Df = dram_D[:]
    for p in range(32):
        bs = [b for b in range(NBLK) if 1 <= 32 * b + p <= N]
        b0, nb = bs[0], len(bs)
        i0 = 32 * b0 + p - 1
        src = _manual_ap(megaRE[4 * p:4 * p + 4,
                                R6_O + b0 * CW + 1:R6_O + b0 * CW + 1 + NB],
                         0, [[CW, nb], [1, NB]])
        dst = bass.AP(df.tensor, i0 * NB, [[N * NB, BC], [32 * NB, nb], [1, NB]])
        nc.sync.dma_start(out=dst, in_=src)

    # ---------------- phase A: forward DP (2 DVE ops per row) ----------------
    # D[oc] = min(mn[oc], D[oc-1]) + d[oc] via tensor_tensor_scan with
    # op0=min, op1=add (state carries D[oc-1]); mn = min(diag, up).
    v.memset(megaQ[0:4, WIN_O:WIN_O + NWIN * 42], BIG)
    v.memset(megaQ[0:4, VR_O:VR_O + 42], BIG)
    v.memset(megaQ[0:4, VR_O + 20:VR_O + 21], 0.0)     # virtual row: D[-1]=0 @ o=20
    v.memset(megaRE[:, R4_O:R4_O + RE], BIG)           # Dre (pads + vrow)
    v.memset(megaRE[0:4, R4_O + 21:R4_O + 22], 0.0)    # vrow in RE (r=0, col 21)

    tmp = megaQ[0:4, TMP_O:TMP_O + NB]
    qbase = megaQ[0:4, 0:1]
    for r in range(1, N + 1):
        i = r - 1
        if i % 32 == 0:
            nrow = min(32, N - i)
            rdst = _manual_ap(qbase, RING_O + (i % NWIN) * NB,
                              [[NB, nrow], [1, NB]])
            rsrc = bass.AP(df.tensor, i * NB, [[N * NB, BC], [NB, nrow], [1, NB]])
            nc.gpsimd.dma_start(out=rdst, in_=rsrc)
        ws = WIN_O + (i % NWIN) * 42
        wp = VR_O if r == 1 else WIN_O + ((i - 1) % NWIN) * 42
        rg0 = RING_O + (i % NWIN) * NB
        dring = megaQ[0:4, rg0:rg0 + NB]
        v.tensor_tensor(out=tmp, in0=megaQ[0:4, wp:wp + NB],
                        in1=megaQ[0:4, wp + 1:wp + NB + 1], op=AL.min)
        v.tensor_tensor_scan(out=megaQ[0:4, ws:ws + NB], data0=tmp,
                             data1=dring, initial=BIG, op0=AL.min, op1=AL.add)
        if i % 32 == 31 or r == N:
            i0 = (i // 32) * 32
            nrow = i - i0 + 1
            k = i // 32
            wsrc = _manual_ap(qbase, WIN_O + (i0 % NWIN) * 42,
                              [[42, nrow], [1, NB]])
            wdst = bass.AP(Df.tensor, i0 * NB, [[N * NB, BC], [NB, nrow], [1, NB]])
            nc.sync.dma_start(out=wdst, in_=wsrc)
            # reload this chunk into Dre right away (overlaps later chunks)
            npp = min(31, N - 1 - 32 * k)          # rows r=32k+1 .. 32k+31
            if npp > 0:
                src = bass.AP(Df.tensor, (32 * k) * NB,
                              [[NB, npp], [N * NB, BC], [1, NB]])
                dst = megaRE[4:4 + 4 * npp, R4_O + k * CW + 1:R4_O + k * CW + 1 + NB]
                nc.sync.dma_start(out=dst, in_=src)
            if 32 * (k + 1) <= N:                  # row r=32(k+1) -> p=0, b=k+1
                src = bass.AP(Df.tensor, (32 * k + 31) * NB,
                              [[N * NB, BC], [1, NB]])
                dst = megaRE[0:4, R4_O + (k + 1) * CW + 1:
                             R4_O + (k + 1) * CW + 1 + NB]
                nc.sync.dma_start(out=dst, in_=src)

    # ---------------- phase B: choice bits + g/L scans ----------------
    v.memset(megaRE[:, R5_O:R5_O + RE], BIG)           # DrePrev
    nc.sync.dma_start(out=megaRE[4:128, R5_O:R5_O + RE],
                      in_=megaRE[0:124, R4_O:R4_O + RE])
    nc.sync.dma_start(out=megaRE[0:4, R5_O + CW:R5_O + RE],
                      in_=megaRE[124:128, R4_O:R4_O + RE - CW])

    diag, up = cells(R5_O, 0), cells(R5_O, 1)
    left = cells(R4_O, -1)
    v.tensor_tensor(out=cells(R7_O), in0=diag, in1=up, op=AL.is_le)
    v.tensor_tensor(out=cells(R8_O), in0=diag, in1=left, op=AL.is_le)
    v.tensor_tensor(out=cells(R8_O), in0=cells(R7_O), in1=cells(R8_O),
                    op=AL.mult)                        # isdiag
    v.tensor_tensor(out=cells(R7_O), in0=left, in1=diag, op=AL.is_lt)
    v.tensor_tensor(out=cells(R6_O), in0=left, in1=up, op=AL.is_lt)
    v.tensor_tensor(out=cells(R7_O), in0=cells(R7_O), in1=cells(R6_O),
                    op=AL.mult)                        # isleft
    v.tensor_single_scalar(out=cells(R6_O), in_=cells(R7_O),
                           scalar=0.0, op=AL.is_equal)  # notleft
    ocp1 = megaRE[:, COLIO_O + 2:COLIO_O + 2 + NB].unsqueeze(1) \
        .broadcast_to([128, NBLK, NB])
    ocol = megaRE[:, COLIO_O + 1:COLIO_O + 1 + NB].unsqueeze(1) \
        .broadcast_to([128, NBLK, NB])
    v.tensor_tensor(out=cells(R8_O), in0=ocp1, in1=cells(R8_O), op=AL.subtract)
    v.tensor_tensor(out=cells(R8_O), in0=cells(R8_O), in1=cells(R6_O),
                    op=AL.mult)                        # gval
    v.tensor_tensor(out=cells(R6_O), in0=ocol, in1=cells(R6_O), op=AL.mult)  # Lval
    for b in range(NBLK):
        v.tensor_tensor_scan(out=reblk(R5_O, b), data0=reblk(R7_O, b),
                             data1=reblk(R8_O, b), initial=0.0,
                             op0=AL.mult, op1=AL.add)  # gfull -> R5
    for b in range(NBLK):
        v.tensor_tensor_scan(out=reblk(R8_O, b), data0=reblk(R7_O, b),
                             data1=reblk(R6_O, b), initial=0.0,
                             op0=AL.mult, op1=AL.add)  # Lfull -> R8

    # ---------------- gwalk copies + walk ----------------
    for half in (1, 0):
        for p in [0] + list(range(31, 0, -1)):     # walk-consumption order
            bs = [b for b in range(NBLK)
                  if 1 <= 32 * b + p <= N
                  and half * 512 <= 32 * b + p - 1 < half * 512 + 512]
            if not bs:
                continue
            b0, nb = bs[0], len(bs)
            i0 = 32 * b0 + p - 1
            src = _manual_ap(
                megaRE[4 * p:4 * p + 4, R5_O + b0 * CW + 1:R5_O + b0 * CW + 1 + NB],
                0, [[CW, nb], [1, NB]])
            q0, go = (0, GWLO_O) if half == 0 else (64, GWHI_O)
            dst = _manual_ap(
                megaQ[q0:q0 + 4, go + (i0 % 512) * NB:go + (i0 % 512) * NB + NB],
                0, [[32 * NB, nb], [1, NB]])
            nc.sync.dma_start(out=dst, in_=src)

    xhl = megaQ[0:4, XHL_O:XHL_O + 512]
    xhh = megaQ[64:68, XHH_O:XHH_O + 513]
    wscl = megaQ[0:4, WSCL_O:WSCL_O + NB]
    wsch = megaQ[64:68, WSCH_O:WSCH_O + NB]
    v.memset(xhh[:, 512:513], 21.0)                    # x_1023 (col coords)
    for i in range(1023, 511, -1):                     # rows 1023..512 (@64)
        g = megaQ[64:68, GWHI_O + (i - 512) * NB:GWHI_O + (i - 512) * NB + NB]
        iot = megaRE[64:68, COLIO_O + 1:COLIO_O + 1 + NB]
        v.scalar_tensor_tensor(out=wsch, in0=iot,
                               scalar=xhh[:, i - 511:i - 510], in1=g,
                               op0=AL.is_equal, op1=AL.mult,
                               accum_out=xhh[:, i - 512:i - 511])
    nc.gpsimd.dma_start(out=xhl[:, 511:512], in_=xhh[:, 0:1])   # x_511
    for i in range(511, 0, -1):                        # rows 511..1 (@0)
        g = megaQ[0:4, GWLO_O + i * NB:GWLO_O + i * NB + NB]
        iot = megaRE[0:4, COLIO_O + 1:COLIO_O + 1 + NB]
        v.scalar_tensor_tensor(out=wscl, in0=iot,
                               scalar=xhl[:, i:i + 1], in1=g,
                               op0=AL.is_equal, op1=AL.mult,
                               accum_out=xhl[:, i - 1:i])

    # ---------------- xcol + olo + mask ----------------
    v.memset(megaRE[:, XC_O:XC_O + NBLK], 0.0)
    for p in range(32):
        for half in (0, 1):
            bs = [b for b in range(NBLK)
                  if 1 <= 32 * b + p <= N
                  and half * 512 <= 32 * b + p - 1 < half * 512 + 512]
            if not bs:
                continue
            b0, nb = bs[0], len(bs)
            i0 = 32 * b0 + p - 1
            if half == 0:
                src = _manual_ap(megaQ[0:4, XHL_O + i0:XHL_O + i0 + 1],
                                 0, [[32, nb]])
            else:
                src = _manual_ap(megaQ[64:68, XHH_O + i0 - 511:XHH_O + i0 - 510],
                                 0, [[32, nb]])
            dst = _manual_ap(megaRE[4 * p:4 * p + 4, XC_O + b0:XC_O + b0 + 1],
                             0, [[1, nb]])
            nc.sync.dma_start(out=dst, in_=src)

    xcolb = smb(XC_O)
    v.tensor_tensor(out=cells(R7_O), in0=ocol, in1=xcolb, op=AL.is_equal)
    v.tensor_tensor(out=cells(R7_O), in0=cells(R7_O), in1=cells(R8_O), op=AL.mult)
    v.tensor_reduce(out=megaRE[:, OLO_O:OLO_O + NBLK], in_=cells(R7_O),
                    axis=mybir.AxisListType.X, op=AL.add)
    v.tensor_tensor(out=cells(R6_O), in0=ocol, in1=smb(OLO_O), op=AL.is_ge)
    v.tensor_tensor(out=cells(R7_O), in0=ocol, in1=xcolb, op=AL.is_le)
    v.tensor_tensor(out=cells(R5_O), in0=cells(R6_O), in1=cells(R7_O),
                    op=AL.mult)                        # mask
    v.memset(megaRE[0:4, R5_O:R5_O + CW], 0.0)         # r=0 virtual slot
    v.memset(megaRE[:, R5_O + 32 * CW:R5_O + 33 * CW], 0.0)  # b=32 junk slots
    # row 1023 (r=1024, p=0, b=32) is real: rebuild its mask (all @0)
    lo1023 = megaRE[0:4, OLO_O + 32:OLO_O + 33]
    hi1023 = megaRE[0:4, XC_O + 32:XC_O + 33]
    ic0 = megaRE[0:4, COLIO_O + 1:COLIO_O + 1 + NB]
    m0 = megaRE[0:4, R5_O + 32 * CW + 1:R5_O + 32 * CW + 1 + NB]
    v.scalar_tensor_tensor(out=wscl, in0=ic0, scalar=lo1023, in1=ic0,
                           op0=AL.is_ge, op1=AL.bypass)
    v.scalar_tensor_tensor(out=m0, in0=ic0, scalar=hi1023, in1=wscl,
                           op0=AL.is_le, op1=AL.mult)

    # ---------------- metrics ----------------
    for src_o, red in ((R1_O, 0), (R2_O, 1)):
        v.tensor_tensor(out=cells(R7_O), in0=cells(src_o), in1=cells(R5_O),
                        op=AL.mult)
        v.tensor_reduce(out=megaRE[:, RED_O + red:RED_O + red + 1],
                        in_=cells(R7_O), axis=mybir.AxisListType.XY, op=AL.add)
    # bce cells: sp(x) + y*(5*sp(-x) - sp(x)),  x = clip(pz, -4, 4)
    v.tensor_scalar(out=megaRE[:, CLZ_O:CLZ_O + NBLK],
                    in0=megaRE[:, PZ_O:PZ_O + NBLK],
                    scalar1=-4.0, scalar2=4.0, op0=AL.max, op1=AL.min)
    nc.scalar.activation(megaRE[:, NGZ_O:NGZ_O + NBLK],
                         megaRE[:, CLZ_O:CLZ_O + NBLK],
                         mybir.ActivationFunctionType.Exp)
    nc.scalar.activation(megaRE[:, SPZ_O:SPZ_O + NBLK],
                         megaRE[:, NGZ_O:NGZ_O + NBLK],
                         mybir.ActivationFunctionType.Ln, bias=1.0)
    nc.scalar.activation(megaRE[:, NGZ_O:NGZ_O + NBLK],
                         megaRE[:, CLZ_O:CLZ_O + NBLK],
                         mybir.ActivationFunctionType.Exp, scale=-1.0)
    nc.scalar.activation(megaRE[:, SPN_O:SPN_O + NBLK],
                         megaRE[:, NGZ_O:NGZ_O + NBLK],
                         mybir.ActivationFunctionType.Ln, bias=1.0)
    v.scalar_tensor_tensor(out=megaRE[:, QZ_O:QZ_O + NBLK],
                           in0=megaRE[:, SPN_O:SPN_O + NBLK], scalar=5.0,
                           in1=megaRE[:, SPZ_O:SPZ_O + NBLK],
                           op0=AL.mult, op1=AL.subtract)
    v.tensor_tensor(out=cells(R7_O), in0=skwin(R3_O), in1=smb(QZ_O), op=AL.mult)
    v.tensor_tensor(out=cells(R7_O), in0=cells(R7_O), in1=smb(SPZ_O), op=AL.add)
    v.tensor_tensor(out=cells(R7_O), in0=cells(R7_O), in1=cells(R5_O), op=AL.mult)
    v.tensor_reduce(out=megaRE[:, RED_O + 2:RED_O + 3], in_=cells(R7_O),
                    axis=mybir.AxisListType.XY, op=AL.add)
    v.tensor_reduce(out=megaRE[:, RED_O + 3:RED_O + 4], in_=cells(R5_O),
                    axis=mybir.AxisListType.XY, op=AL.add)

    nc.sync.dma_start(out=partials[:], in_=megaRE[:, RED_O:RED_O + 4])


def _get_module():
    if "nc" not in _CACHE:
        _CACHE["nc"] = _build_module()
    return _CACHE["nc"]


def _make_inmaps(preds, targs):
    preds = np.ascontiguousarray(preds, dtype=np.float32)
    targs = np.ascontiguousarray(targs, dtype=np.float32)
    cst = np.zeros((128, 78), dtype=np.float32)
    cst[:, 0] = np.arange(128) // 4
    cst[:, 1:44] = np.arange(43)[None, :]
    cst[:, 44:77] = (32 * np.arange(NBLK)[None, :]
                     + (np.arange(128) // 4)[:, None] - 22)
    pp = np.arange(32)
    bb = np.arange(NBLK)
    r_idx = pp[:, None] + 32 * bb[None, :]              # [32, 33]
    r_ok = (r_idx >= 1) & (r_idx <= N)
    r_cl = np.clip(r_idx - 1, 0, N - 1)
    uu = np.arange(SKW)
    t_idx = uu[None, :] + pp[:, None] - 21              # [32, SKW]
    t_ok = (t_idx >= 0) & (t_idx < N)
    t_cl = np.clip(t_idx, 0, N - 1)
    in_maps = []
    for c in range(NCORES):
        ps = preds[c * BC:(c + 1) * BC]
        ts = targs[c * BC:(c + 1) * BC]
        pre = np.zeros((32, BC, 3 * NBLK), dtype=np.float32)
        tskv = np.zeros((32, BC, 3 * SKW), dtype=np.float32)
        for k in range(3):
            vv = ps[:, :, k][:, r_cl]                   # [BC, 32, NBLK]
            pre[:, :, k * NBLK:(k + 1) * NBLK] = \
                np.where(r_ok[None], vv, 0.0).transpose(1, 0, 2)
            ww = ts[:, :, k][:, t_cl]                   # [BC, 32, SKW]
            tskv[:, :, k * SKW:(k + 1) * SKW] = \
                np.where(t_ok[None], ww, 0.0).transpose(1, 0, 2)
        in_maps.append({"pre": pre.reshape(128, 3 * NBLK),
                        "tsk": tskv.reshape(128, 3 * SKW), "cst": cst})
    return in_maps


def _reduce_host(parts_list, subcoef):
    c0, c1 = float(subcoef[0]), float(subcoef[1])
    loss = 0.0
    for parts in parts_list:
        m = parts.reshape(32, BC, 4).sum(axis=0)        # [BC, (Sx,Sy,Sb,cnt)]
        for s in range(BC):
            sx, sy, sb, cnt = (float(m[s, k]) for k in range(4))
            loss += c0 * sx + c1 * sy + 0.1 * sb / cnt
    return np.float32(loss)


def run(preds, targs, subcoef, trace=False):
    nc = _get_module()
    in_maps = _make_inmaps(preds, targs)
    res = run_bass_kernel_spmd(nc, in_maps, core_ids=list(range(NCORES)),
                               trace=trace)
    parts = [r["partials"] for r in res.results]
    return _reduce_host(parts, np.asarray(subcoef)), res


def kernel(preds, targs, subcoef):
    out, _ = run(preds, targs, subcoef)
    return out



# revision 67
# speedup vs baseline: 2.0713x; 2.0713x over previous
"""Banded DTW loss kernel for Trainium2 (Bass/Tile), 8-core data-parallel.

Bidirectional (meet-in-the-middle) formulation, v2:
  The 1024-row banded DP is split into two 512-row halves that run
  SIMULTANEOUSLY in the same [8,41] DVE ops: lanes 0:4 = forward DP of
  rows 0..511 (4 samples), lanes 4:8 = forward DP of the REVERSED
  sequences (= reverse DP of rows 1023..512, mirrored coords).  This
  halves both serial chains (512 DP iterations of 2 ops, 511 walk steps
  of 1 op).  Any cost-optimal DTW path gives the same Sx+Sy (== the DP
  optimum); only the tiny BCE/cnt term depends on tie-breaking, so exact
  reference tie-order is not required.

  Phase A: 512 rows x (min + tensor_tensor_scan) on [8,41]; rolling
           128-slot D window + 256-slot d ring; one plain-slice DMA per
           32-row block stages d in (ring) and D out (RE), interleaved
           with the row loop so nothing stalls.
  Meet:    total(o) = F[511,o] + min(R[512,o-1], R[512,o]); first argmin
           picks the crossing; seeds both walks.
  Phase B: choice bits + g/L scans in an RE layout of 32 blocks
           (partition = 32*sample + (rv-1)%32, block = (rv-1)//32;
           F rows at blocks 0..15, mirrored-R rows at blocks 16..31;
           virtual boundary rows are memset directly into the DrePrev
           copy at (p=0, b=0/16) and survive the shift DMAs).
  Walk:    511 shared one-hot extract steps on [8,41]; per-block g
           tables DMA'd in walk order so the stream stays ahead.
  Masks/metrics: interval masks from (lo, x) per row; 4 aggregates
           reduced per partition; host sums partials.

Sharding: batch 32 -> 4 samples per core on 8 cores; host sums partials.
"""

import numpy as np

import concourse.bacc as bacc
import concourse.bass as bass
import concourse.mybir as mybir
import concourse.tile as tile
from concourse.bass_utils import run_bass_kernel_spmd

B, N, NF = 32, 1024, 4
W = 20
NCORES = 8
BC = B // NCORES          # samples per core
BIG = 1e30
NB = 41                   # band width
CW = 43                   # RE row width (col 0 pad, col c=o+1, col 42 pad)
NBLK = 32                 # RE blocks (F: b 0..15, R': b 16..31)
HBLK = 16
H = N // 2                # 512 DP rows per half
RING = 256                # d ring slots (41 wide)
NWIN = 128                # D window slots (42 wide, col 41 = BIG pad)
SKW2 = 522                # skew array width per half
RSK = 560                 # R'-half skew offset inside R1/R2/R3 regions

AL = mybir.AluOpType
DT = mybir.dt.float32
AX = mybir.AxisListType

# ---- megaQ ([128, QW]) regions; DP lanes live on partitions 0:8 ----
WIN_O = 0                          # 128 slots * 42
VR_O = WIN_O + NWIN * 42           # virtual row (42)
RING_O = VR_O + 42                 # 256 slots * 41
TMP_O = RING_O + RING * NB         # 48
WSC_O = TMP_O + 48                 # walk scratch (48)
XF_O = WSC_O + 48                  # xfull (516)
DUMP_O = XF_O + 516                # spill for p=0 rows of gwalk blocks (41)
GW_O = DUMP_O + NB                 # walk g tables: 511 * 41
MEET_O = GW_O + 511 * NB           # meet scratch
MRR_O = MEET_O                     # 43: Rreal padded (col0 BIG)
MTV_O, MTD_O, MTO_O, MWS_O = (MEET_O + 43 + i * NB for i in range(4))
MSC_O = MWS_O + NB                 # scalars: MN, OS, TVS, TDS, VF, T1
QW = MSC_O + 16

# ---- megaRE ([128, REW]) regions ----
RE = NBLK * CW                     # 1376
R1_O, R2_O, R3_O, R4_O, R5_O, R6_O, R7_O, R8_O = (i * RE for i in range(8))
SM_O = 8 * RE
PX_O, PY_O, PZ_O = SM_O, SM_O + NBLK, SM_O + 2 * NBLK
XC_O, OLO_O = SM_O + 3 * NBLK, SM_O + 4 * NBLK
COLIO_O = SM_O + 5 * NBLK          # 43 values 0..42
DESC_O = COLIO_O + 43              # 41 values 41..1
CLZ_O = DESC_O + 41
SPZ_O, SPN_O, QZ_O = CLZ_O + NBLK, CLZ_O + 2 * NBLK, CLZ_O + 3 * NBLK
RED_O = CLZ_O + 4 * NBLK           # Sx, Sy, Sbce, cnt
REW = RED_O + 8

_CACHE = {}


def _ap(t, part0, off, dims):
    """AP at partition `part0`, col offset `off`, explicit [stride,count]
    dims (strides in elements; partition stride = tile pitch)."""
    base = t[part0:part0 + 1, 0:1]
    return bass.AP(base.tensor, base.offset + off, [list(d) for d in dims])


def _build_module():
    nc = bacc.Bacc("TRN2", target_bir_lowering=False, debug=False,
                   num_devices=NCORES)
    pre = nc.dram_tensor("pre", [128, 3 * NBLK], DT, kind="ExternalInput")
    tsk = nc.dram_tensor("tsk", [128, 6 * SKW2], DT, kind="ExternalInput")
    cst = nc.dram_tensor("cst", [128, 84], DT, kind="ExternalInput")
    partials = nc.dram_tensor("partials", [128, 4], DT, kind="ExternalOutput")

    with tile.TileContext(nc) as tc:
        with tc.tile_pool(name="main", bufs=1) as pool:
            megaQ = pool.tile([128, QW], DT)
            megaRE = pool.tile([128, REW], DT)
            _emit(nc, megaQ, megaRE, pre, tsk, cst, partials)
    nc.compile()
    return nc


def _emit(nc, megaQ, megaRE, pre, tsk, cst, partials):
    v = nc.vector
    QP = QW      # megaQ partition pitch
    RP = REW     # megaRE partition pitch

    def cells(off, dc=0, b0=0, nb=NBLK):
        s = off + b0 * CW
        return megaRE[:, s:s + nb * CW].rearrange(
            "p (b c) -> p b c", c=CW)[:, :, 1 + dc:NB + 1 + dc]

    def smb(off, b0=0, nb=NBLK):
        return megaRE[:, off + b0:off + b0 + nb].unsqueeze(2) \
            .broadcast_to([128, nb, NB])

    def ocolv(shift=0, nb=NBLK):
        s = COLIO_O + 1 + shift
        return megaRE[:, s:s + NB].unsqueeze(1).broadcast_to([128, nb, NB])

    def skwin(off, skb, nb):
        base = megaRE[:, off + skb:off + skb + 1]
        ap0 = [list(base.ap[0])]
        return bass.AP(base.tensor, base.offset,
                       ap0 + [[32, nb], [1, NB]])

    # ---------------- input DMAs ----------------
    # Ordered by first use on the serialized HWDGE: the d build's first
    # op needs the tx skews + pre; cst (iota tables) and the tz skew
    # (BCE cells under phase A) come last.
    nc.sync.dma_start(out=megaRE[:, R1_O:R1_O + SKW2], in_=tsk[:, 0:SKW2])
    nc.sync.dma_start(out=megaRE[:, R1_O + RSK:R1_O + RSK + SKW2],
                      in_=tsk[:, 3 * SKW2:4 * SKW2])
    nc.sync.dma_start(out=megaRE[:, PX_O:PX_O + 3 * NBLK], in_=pre[:])
    nc.sync.dma_start(out=megaRE[:, R2_O:R2_O + SKW2],
                      in_=tsk[:, SKW2:2 * SKW2])
    nc.sync.dma_start(out=megaRE[:, R2_O + RSK:R2_O + RSK + SKW2],
                      in_=tsk[:, 4 * SKW2:5 * SKW2])
    nc.sync.dma_start(out=megaRE[:, COLIO_O:COLIO_O + 84], in_=cst[:])
    nc.sync.dma_start(out=megaRE[:, R3_O:R3_O + SKW2],
                      in_=tsk[:, 2 * SKW2:3 * SKW2])
    nc.sync.dma_start(out=megaRE[:, R3_O + RSK:R3_O + RSK + SKW2],
                      in_=tsk[:, 5 * SKW2:6 * SKW2])

    # ---------------- init memsets ----------------
    v.memset(_ap(megaQ, 0, WIN_O + 41, [[QP, 8], [42, NWIN]]), BIG)  # win pads
    v.memset(megaQ[0:8, VR_O:VR_O + 42], BIG)
    v.memset(megaQ[0:8, VR_O + 20:VR_O + 21], 0.0)       # DP origin (o=20)
    v.memset(megaQ[0:4, MRR_O:MRR_O + 1], BIG)           # meet pad
    v.memset(_ap(megaRE, 0, R4_O, [[RP, 128], [CW, NBLK]]), BIG)      # pads
    v.memset(_ap(megaRE, 0, R4_O + 42, [[RP, 128], [CW, NBLK]]), BIG)
    v.memset(_ap(megaRE, 0, R5_O, [[RP, 128], [CW, NBLK]]), BIG)
    v.memset(_ap(megaRE, 0, R5_O + 42, [[RP, 128], [CW, NBLK]]), BIG)
    v.memset(megaRE[:, XC_O:XC_O + NBLK], 0.0)

    # ---------------- BCE scalar prep (Act engine; runs under phase A) ----
    v.tensor_scalar(out=megaRE[:, CLZ_O:CLZ_O + NBLK],
                    in0=megaRE[:, PZ_O:PZ_O + NBLK],
                    scalar1=-4.0, scalar2=4.0, op0=AL.max, op1=AL.min)
    nc.scalar.activation(megaRE[:, SPN_O:SPN_O + NBLK],
                         megaRE[:, CLZ_O:CLZ_O + NBLK],
                         mybir.ActivationFunctionType.Exp)
    nc.scalar.activation(megaRE[:, SPZ_O:SPZ_O + NBLK],
                         megaRE[:, SPN_O:SPN_O + NBLK],
                         mybir.ActivationFunctionType.Ln, bias=1.0)
    nc.scalar.activation(megaRE[:, QZ_O:QZ_O + NBLK],
                         megaRE[:, CLZ_O:CLZ_O + NBLK],
                         mybir.ActivationFunctionType.Exp, scale=-1.0)
    nc.scalar.activation(megaRE[:, SPN_O:SPN_O + NBLK],
                         megaRE[:, QZ_O:QZ_O + NBLK],
                         mybir.ActivationFunctionType.Ln, bias=1.0)
    v.scalar_tensor_tensor(out=megaRE[:, QZ_O:QZ_O + NBLK],
                           in0=megaRE[:, SPN_O:SPN_O + NBLK], scalar=5.0,
                           in1=megaRE[:, SPZ_O:SPZ_O + NBLK],
                           op0=AL.mult, op1=AL.subtract)

    # ---------------- d build ----------------
    # Band validity needs no explicit mask: the host poisons out-of-range
    # target x/y values with 5e14, so d at invalid cells is ~1e15 (BIG-
    # like for the DP) while valid cells get exact |dx|+|dy|.
    for b0, skb in ((0, 0), (HBLK, RSK)):
        v.tensor_tensor(out=cells(R5_O, 0, b0, HBLK), in0=smb(PX_O, b0, HBLK),
                        in1=skwin(R1_O, skb, HBLK), op=AL.subtract)
    v.scalar_tensor_tensor(out=cells(R1_O), in0=cells(R5_O), scalar=-1.0,
                           in1=cells(R5_O), op0=AL.mult, op1=AL.max)
    for b0, skb in ((0, 0), (HBLK, RSK)):
        v.tensor_tensor(out=cells(R5_O, 0, b0, HBLK), in0=smb(PY_O, b0, HBLK),
                        in1=skwin(R2_O, skb, HBLK), op=AL.subtract)
    v.scalar_tensor_tensor(out=cells(R2_O), in0=cells(R5_O), scalar=-1.0,
                           in1=cells(R5_O), op0=AL.mult, op1=AL.max)
    v.tensor_tensor(out=cells(R6_O), in0=cells(R1_O), in1=cells(R2_O),
                    op=AL.add)                         # dcost -> R6
    # virtual boundary rows for DrePrev: p=0 partitions only (quadrant
    # starts are legal DVE start partitions); nothing else writes them.
    for s in range(4):
        p0 = 32 * s
        for hb in (0, HBLK):
            v.memset(megaRE[p0:p0 + 1,
                            R5_O + hb * CW:R5_O + hb * CW + CW], BIG)
            v.memset(megaRE[p0:p0 + 1,
                            R5_O + hb * CW + 21:R5_O + hb * CW + 22], 0.0)
    v.memset(megaRE[:, RED_O + 1:RED_O + 2], 0.0)      # unused Sy slot

    # ---------------- staging DMAs (per 32-row block) ----------------
    def dstage(b):
        """d for rows rv=32b+1..32b+32 (block b F, b+16 R') -> ring."""
        for hb, lane in ((0, 0), (HBLK, 4)):
            nc.sync.dma_start(
                out=_ap(megaQ, lane, RING_O + ((32 * b) % RING) * NB,
                        [[QP, 4], [NB, 32], [1, NB]]),
                in_=megaRE[:, R6_O + (hb + b) * CW + 1:
                           R6_O + (hb + b) * CW + 1 + NB])

    def evac(b):
        """window slots for rows r=32b+1..32b+32 -> R4 block b / b+16."""
        s0 = WIN_O + ((32 * b) % NWIN) * 42
        for hb, lane in ((0, 0), (HBLK, 4)):
            # SP queue: lower DGE start latency (650 vs 784 ns) on the
            # evac(15) -> choice-fast critical edge
            nc.sync.dma_start(
                out=megaRE[:, R4_O + (hb + b) * CW + 1:
                           R4_O + (hb + b) * CW + 1 + NB],
                in_=_ap(megaQ, lane, s0, [[QP, 4], [42, 32], [1, NB]]))

    for b in range(8):
        dstage(b)

    # ---------------- phase A: 512 rows, 2 DVE ops each ----------------
    tmp8 = megaQ[0:8, TMP_O:TMP_O + NB]
    for r in range(1, H + 1):
        wp = VR_O if r == 1 else WIN_O + ((r - 2) % NWIN) * 42
        ws = WIN_O + ((r - 1) % NWIN) * 42
        rc = RING_O + ((r - 1) % RING) * NB
        v.tensor_tensor(out=tmp8, in0=megaQ[0:8, wp:wp + NB],
                        in1=megaQ[0:8, wp + 1:wp + NB + 1], op=AL.min)
        v.tensor_tensor_scan(out=megaQ[0:8, ws:ws + NB], data0=tmp8,
                             data1=megaQ[0:8, rc:rc + NB], initial=BIG,
                             op0=AL.min, op1=AL.add)
        if r % 32 == 0:
            bdone = r // 32 - 1
            evac(bdone)
            if bdone + 8 < HBLK:
                dstage(bdone + 8)

    # ---------------- meet ----------------
    w511 = WIN_O + 127 * 42
    f511 = megaQ[0:4, w511:w511 + NB]
    nc.sync.dma_start(
        out=megaQ[0:4, MRR_O + 1:MRR_O + 42],
        in_=_ap(megaQ, 4, w511 + 40, [[QP, 4], [-1, NB]]))
    tv = megaQ[0:4, MTV_O:MTV_O + NB]
    td = megaQ[0:4, MTD_O:MTD_O + NB]
    tot = megaQ[0:4, MTO_O:MTO_O + NB]
    mws = megaQ[0:4, MWS_O:MWS_O + NB]
    mn = megaQ[0:4, MSC_O:MSC_O + 1]
    osr = megaQ[0:4, MSC_O + 1:MSC_O + 2]
    tvs = megaQ[0:4, MSC_O + 2:MSC_O + 3]
    tds = megaQ[0:4, MSC_O + 3:MSC_O + 4]
    vf = megaQ[0:4, MSC_O + 4:MSC_O + 5]
    t1 = megaQ[0:4, MSC_O + 5:MSC_O + 6]
    iota0 = megaRE[0:4, COLIO_O:COLIO_O + NB]            # 0..40
    desc = megaRE[0:4, DESC_O:DESC_O + NB]               # 41..1
    v.tensor_tensor(out=tv, in0=f511, in1=megaQ[0:4, MRR_O:MRR_O + NB],
                    op=AL.add)
    v.tensor_tensor(out=td, in0=f511, in1=megaQ[0:4, MRR_O + 1:MRR_O + 42],
                    op=AL.add)
    v.tensor_tensor(out=tot, in0=tv, in1=td, op=AL.min)
    v.tensor_reduce(out=mn, in_=tot, axis=AX.X, op=AL.min)
    v.scalar_tensor_tensor(out=mws, in0=tot, scalar=mn, in1=desc,
                           op0=AL.is_equal, op1=AL.mult)
    v.tensor_reduce(out=osr, in_=mws, axis=AX.X, op=AL.max)
    v.tensor_scalar(out=osr, in0=osr, scalar1=-1.0, scalar2=41.0,
                    op0=AL.mult, op1=AL.add)             # o*
    v.scalar_tensor_tensor(out=mws, in0=iota0, scalar=osr, in1=tv,
                           op0=AL.is_equal, op1=AL.mult, accum_out=tvs)
    # vertical iff tv achieves the min at o* (tot[o*] == mn exactly)
    v.tensor_tensor(out=vf, in0=tvs, in1=mn, op=AL.is_equal)
    v.tensor_scalar(out=megaQ[0:4, XF_O + 511:XF_O + 512], in0=osr,
                    scalar1=1.0, scalar2=0.0, op0=AL.add, op1=AL.add)
    v.tensor_tensor(out=t1, in0=vf, in1=osr, op=AL.subtract)
    v.tensor_scalar(out=t1, in0=t1, scalar1=41.0, scalar2=0.0,
                    op0=AL.add, op1=AL.add)              # 41 - o* + vf
    nc.sync.dma_start(out=_ap(megaQ, 4, XF_O + 511, [[QP, 4], [1, 1]]),
                      in_=_ap(megaQ, 0, MSC_O + 5, [[QP, 4], [1, 1]]))

    # ---------------- BCE cells (mask-independent; fills DrePrev wait) --
    for b0, skb in ((0, 0), (HBLK, RSK)):
        v.tensor_tensor(out=cells(R1_O, 0, b0, HBLK),
                        in0=skwin(R3_O, skb, HBLK),
                        in1=smb(QZ_O, b0, HBLK), op=AL.mult)
        v.tensor_tensor(out=cells(R1_O, 0, b0, HBLK),
                        in0=cells(R1_O, 0, b0, HBLK),
                        in1=smb(SPZ_O, b0, HBLK), op=AL.add)

    # ---------------- phase B: DrePrev, choice bits, g/L scans ----------
    # R5 slot (p,b) = D of rv-1.  Per-sample partition shifts (31-wide, so
    # the p=0 partitions keep the early-memset virtual rows), plus p=0
    # fixups for b>=1 from the previous block's p=31 row.  Everything is
    # chunked by block range, high blocks first, so the walk's dependency
    # chain only runs through blocks 13..15/29..31; Tile streams the rest
    # underneath the walk.
    QS = (nc.scalar, nc.sync, nc.gpsimd)

    def dreprev(blo, bhi):
        w = (bhi - blo + 1) * CW
        for s in range(4):
            p0 = 32 * s
            for i, hb in enumerate((0, HBLK)):
                base = R5_O + (hb + blo) * CW
                src = R4_O + (hb + blo) * CW
                QS[(2 * s + i) % 3].dma_start(
                    out=megaRE[p0 + 1:p0 + 32, base:base + w],
                    in_=megaRE[p0:p0 + 31, src:src + w])
                flo = max(blo, 1) if hb == 0 else max(blo, 1 + HBLK) - HBLK
                if flo <= bhi:
                    fw = (bhi - flo + 1) * CW
                    fb = R5_O + (hb + flo) * CW
                    fs = R4_O + (hb + flo - 1) * CW
                    QS[(2 * s + i + 1) % 3].dma_start(
                        out=megaRE[p0:p0 + 1, fb:fb + fw],
                        in_=megaRE[p0 + 31:p0 + 32, fs:fs + fw])

    def choice(blo, nb):
        # isleft -> R7, isdiag/gval -> R8, notleft -> R2, Lval -> R3
        for b0 in (blo, HBLK + blo):
            diag = cells(R5_O, 0, b0, nb)
            up = cells(R5_O, 1, b0, nb)
            left = cells(R4_O, -1, b0, nb)
            c2 = cells(R2_O, 0, b0, nb)
            c3 = cells(R3_O, 0, b0, nb)
            c7 = cells(R7_O, 0, b0, nb)
            c8 = cells(R8_O, 0, b0, nb)
            v.tensor_tensor(out=c2, in0=diag, in1=up, op=AL.min)
            v.tensor_tensor(out=c7, in0=left, in1=c2, op=AL.is_lt)  # isleft
            v.tensor_tensor(out=c2, in0=left, in1=up, op=AL.min)
            v.tensor_tensor(out=c8, in0=diag, in1=c2, op=AL.is_le)  # isdiag
            v.tensor_single_scalar(out=c2, in_=c7, scalar=0.0,
                                   op=AL.is_equal)                  # notleft
            v.tensor_tensor(out=c3, in0=ocolv(1, nb), in1=c2, op=AL.mult)
            v.tensor_tensor(out=c8, in0=c3, in1=c8, op=AL.subtract)  # gval
            v.tensor_tensor(out=c3, in0=c3, in1=c2, op=AL.subtract)  # Lval

    # Fast path for blocks 13..15/29..31: their DrePrev rows (D rows
    # 416..511) are still live in the window ring (slots 31..126), which
    # is row-linear, so one plain DMA per (block, half) covers all p
    # including p=0 — and it does not wait on the evac hop.
    for i, b in enumerate((15, 14, 13)):
        s0 = WIN_O + ((32 * b - 1) % NWIN) * 42
        for hb, lane in ((0, 0), (HBLK, 4)):
            QS[(2 * i + (hb > 0)) % 3].dma_start(
                out=megaRE[:, R5_O + (hb + b) * CW + 1:
                           R5_O + (hb + b) * CW + 1 + NB],
                in_=_ap(megaQ, lane, s0, [[QP, 4], [42, 32], [1, NB]]))
    choice(13, 3)
    dreprev(7, 12)
    dreprev(0, 6)
    choice(7, 6)
    choice(0, 7)
    # g scans + gwalk DMAs in DESCENDING block order so the walk (which
    # consumes gw slots high-k first) can start as soon as pair 15/31 is
    # staged; the rest streams underneath it.  gw slot col (k-1)*41 holds
    # g of row k: F rv=k+1 at (p=k%32, b=k//32), R' at b=16+k//32; the
    # unused p=0 row of block 0 lands in the DUMP spill region.
    for b in range(HBLK - 1, -1, -1):
        for hb in (0, HBLK):
            v.tensor_tensor_scan(
                out=megaRE[:, R5_O + (hb + b) * CW + 1:
                           R5_O + (hb + b) * CW + 1 + NB],
                data0=megaRE[:, R7_O + (hb + b) * CW + 1:
                             R7_O + (hb + b) * CW + 1 + NB],
                data1=megaRE[:, R8_O + (hb + b) * CW + 1:
                             R8_O + (hb + b) * CW + 1 + NB],
                initial=0.0, op0=AL.mult, op1=AL.add)    # gfull -> R5
        for hb, lane in ((0, 0), (HBLK, 4)):
            nc.gpsimd.dma_start(
                out=_ap(megaQ, lane, GW_O + (32 * b - 1) * NB,
                        [[QP, 4], [NB, 32], [1, NB]]),
                in_=megaRE[:, R5_O + (hb + b) * CW + 1:
                           R5_O + (hb + b) * CW + 1 + NB])

    # ---------------- per-chunk mask + metric cells -------------------
    # Everything except three final reductions is computed per 4-block
    # chunk as soon as the walk has produced that chunk's xcol columns,
    # so it streams into the walk's dependency-latency gaps.
    # ---------------- walk: 511 shared steps + streamed extras ---------
    # xcol block b (xfull cols 32b..32b+31 -> XC col b / b+16) is emitted
    # as soon as walk step k=32b+1 has produced col 32b; tail chunk j
    # follows once its 4 blocks' xcols are all emitted.
    iot8 = megaRE[0:8, COLIO_O + 1:COLIO_O + 1 + NB]
    wsc8 = megaQ[0:8, WSC_O:WSC_O + NB]
    for k in range(H - 1, 0, -1):
        v.scalar_tensor_tensor(
            out=wsc8, in0=iot8, scalar=megaQ[0:8, XF_O + k:XF_O + k + 1],
            in1=megaQ[0:8, GW_O + (k - 1) * NB:GW_O + k * NB],
            op0=AL.is_equal, op1=AL.mult,
            accum_out=megaQ[0:8, XF_O + k - 1:XF_O + k])
        if k % 32 == 1:
            b = k // 32
            # the last (b=0) scatter gates the olo/mask tail: use the
            # lower-latency SP HWDGE path for it instead of Pool SWDGE
            xq = nc.sync if b == 0 else nc.gpsimd
            for lane, hb in ((0, 0), (4, HBLK)):
                xq.dma_start(
                    out=megaRE[:, XC_O + hb + b:XC_O + hb + b + 1],
                    in_=megaQ[lane:lane + 4, XF_O + 32 * b:XF_O + 32 * b + 32])
    for bb in range(NBLK):
        v.tensor_tensor_scan(
            out=megaRE[:, R8_O + bb * CW + 1:R8_O + bb * CW + 1 + NB],
            data0=megaRE[:, R7_O + bb * CW + 1:R7_O + bb * CW + 1 + NB],
            data1=megaRE[:, R3_O + bb * CW + 1:R3_O + bb * CW + 1 + NB],
            initial=0.0, op0=AL.mult, op1=AL.add)        # Lfull -> R8

    # ---------------- olo + mask + metrics (bulk) ----------------
    # Sxy = sum(dcost * mask): inside the mask every cell is band-valid,
    # so dcost == |dx|+|dy| there (subcoef is ones per the input spec, so
    # Sx and Sy need not be separated).  BCE cells were precomputed in R1.
    xcolb = smb(XC_O)
    v.tensor_tensor(out=cells(R7_O), in0=ocolv(0), in1=xcolb, op=AL.is_equal)
    v.tensor_tensor(out=cells(R7_O), in0=cells(R7_O), in1=cells(R8_O),
                    op=AL.mult)
    v.tensor_reduce(out=megaRE[:, OLO_O:OLO_O + NBLK], in_=cells(R7_O),
                    axis=AX.X, op=AL.add)
    v.tensor_tensor(out=cells(R2_O), in0=ocolv(0), in1=smb(OLO_O),
                    op=AL.is_ge)
    v.tensor_tensor(out=cells(R7_O), in0=ocolv(0), in1=xcolb, op=AL.is_le)
    v.tensor_tensor(out=cells(R5_O), in0=cells(R2_O), in1=cells(R7_O),
                    op=AL.mult)                          # mask -> R5
    v.tensor_tensor(out=cells(R7_O), in0=cells(R6_O), in1=cells(R5_O),
                    op=AL.mult)
    v.tensor_reduce(out=megaRE[:, RED_O:RED_O + 1], in_=cells(R7_O),
                    axis=AX.XY, op=AL.add)
    v.tensor_tensor(out=cells(R2_O), in0=cells(R1_O), in1=cells(R5_O),
                    op=AL.mult)
    v.tensor_reduce(out=megaRE[:, RED_O + 2:RED_O + 3], in_=cells(R2_O),
                    axis=AX.XY, op=AL.add)
    # cnt closed form: per-row run length is x - lo + 1, so sum the tiny
    # [128,32] difference instead of reducing the full mask cells (host
    # adds the +1-per-row constant).
    v.tensor_tensor(out=megaRE[:, CLZ_O:CLZ_O + NBLK],
                    in0=megaRE[:, XC_O:XC_O + NBLK],
                    in1=megaRE[:, OLO_O:OLO_O + NBLK], op=AL.subtract)
    v.tensor_reduce(out=megaRE[:, RED_O + 3:RED_O + 4],
                    in_=megaRE[:, CLZ_O:CLZ_O + NBLK], axis=AX.X, op=AL.add)

    nc.sync.dma_start(out=partials[:], in_=megaRE[:, RED_O:RED_O + 4])


def _make_inmaps(preds, targs):
    preds = np.ascontiguousarray(preds, dtype=np.float32)
    targs = np.ascontiguousarray(targs, dtype=np.float32)
    pp = np.arange(32)
    bb = np.arange(NBLK)
    # row index per (p, b): F blocks b<16: i = 32b+p; R': i' = 32(b-16)+p
    iF = 32 * bb[None, :16] + pp[:, None]                # [32, 16]
    iR = 32 * (bb[None, 16:] - HBLK) + pp[:, None]
    idx = np.concatenate([iF, N - 1 - iR], axis=1)       # [32, 32] real rows

    uu = np.arange(SKW2)
    tF = uu[None, :] + pp[:, None] - 20                  # [32, SKW2]
    okF = (tF >= 0) & (tF < N)
    tFc = np.clip(tF, 0, N - 1)
    tR = 1043 - uu[None, :] - pp[:, None]
    okR = (tR >= 0) & (tR < N)
    tRc = np.clip(tR, 0, N - 1)

    cstrow = np.concatenate([np.arange(43),
                             np.arange(41, 0, -1)]).astype(np.float32)
    cstf = np.repeat(cstrow[None], 128, axis=0).copy()

    in_maps = []
    for c in range(NCORES):
        ps = preds[c * BC:(c + 1) * BC]                  # [4, N, F]
        ts = targs[c * BC:(c + 1) * BC]
        prev = np.zeros((4, 32, 3 * NBLK), np.float32)
        tskv = np.zeros((4, 32, 6 * SKW2), np.float32)
        for k in range(3):
            # poison out-of-range x/y targets so d is ~1e15 there (band
            # validity without an explicit mask); z stays 0 (masked out)
            pz = 0.0 if k == 2 else 5e14
            prev[:, :, k * NBLK:(k + 1) * NBLK] = ps[:, :, k][:, idx]
            tskv[:, :, k * SKW2:(k + 1) * SKW2] = \
                np.where(okF[None], ts[:, :, k][:, tFc], pz)
            tskv[:, :, (3 + k) * SKW2:(4 + k) * SKW2] = \
                np.where(okR[None], ts[:, :, k][:, tRc], pz)
        in_maps.append({"pre": prev.reshape(128, 3 * NBLK),
                        "tsk": tskv.reshape(128, 6 * SKW2), "cst": cstf})
    return in_maps


def _reduce_host(parts_list, subcoef):
    c0, c1 = float(subcoef[0]), float(subcoef[1])
    loss = 0.0
    for parts in parts_list:
        m = parts.reshape(BC, 32, 4).sum(axis=1)         # [s, (Sx,Sy,Sb,cnt)]
        for s in range(BC):
            sx, sy, sb, cnt = (float(m[s, k]) for k in range(4))
            loss += c0 * sx + c1 * sy + 0.1 * sb / (cnt + N)
    return np.float32(loss)


def _get_module():
    if "nc" not in _CACHE:
        _CACHE["nc"] = _build_module()
    return _CACHE["nc"]


def run(preds, targs, subcoef, trace=False):
    nc = _get_module()
    in_maps = _make_inmaps(preds, targs)
    res = run_bass_kernel_spmd(nc, in_maps, core_ids=list(range(NCORES)),
                               trace=trace)
    parts = [r["partials"] for r in res.results]
    return _reduce_host(parts, np.asarray(subcoef)), res


def kernel(preds, targs, subcoef):
    out, _ = run(preds, targs, subcoef)
    return out
